# revision 34
# baseline (speedup 1.0000x reference)
"""Block-causal self-attention (SSMax) Trainium2 kernel, v2 (pipelined).

Full inputs in, full output out. Sharding: 8 cores = 2 batches x 4 head
groups (3 heads each). Each core computes qkv for its head slice, the
block-causal attention for its 3 heads, and a partial c_proj product;
the host sums the 4 partials per batch.

v2 layout/schedule notes (per core):
  - x shipped twice: xt f32r [768,2048] feeds the q/k projection (f32r
    streams 1 cycle/row at >=256 moving cols, so no precision loss is
    paid for q/k), xtb bf16 feeds the v projection where lhsT=x chunks
    produce v directly in [token, dim] layout (no PE transposes).
  - Everything is software-pipelined per 512-token chunk c:
    qkv(c) -> scores/exp/y(c) -> proj(c), with attention group c
    consuming only k/v chunks <= c, so the Scalar-engine exp overlaps
    the whole run instead of serializing behind qkv.
  - Scores are computed transposed (ST[j,i] = k_j . q_i) in [128,1024]
    f32 PSUM pair-tiles (two 128-key chunks per tile); one exp per tile.
    q columns are pre-scaled by s*log(T)/sqrt(hd) on the host.
  - y accumulation chains (one per query chunk r x head) stay open
    across pair-tiles and are fed immediately after each pair's exp,
    keeping PE and ACT in lockstep. The softmax denominator comes from
    a ones-column appended to v (col 64 of each head's 65-col block).
  - Projection output is staged [0:512] via DVE and [0:256] via the
    Scalar engine to balance the two engines; partials ship bf16.
"""

import numpy as np

T = 2048
C = 768
HD = 64
KC = 6  # 768 / 128 contraction chunks
N_CORES = 8

_CACHE: dict = {}


def _build_bass():
    import concourse.bacc as bacc
    import concourse.mybir as mybir
    import concourse.tile as tile
    from concourse._compat import get_trn_type
    from concourse.masks import make_identity

    dt = mybir.dt
    f32 = dt.float32
    f32r = dt.float32r
    bf16 = dt.bfloat16
    EXP = mybir.ActivationFunctionType.Exp
    COPY = mybir.ActivationFunctionType.Copy

    nc = bacc.Bacc(get_trn_type() or "TRN2", debug=False)
    xtb_d = nc.dram_tensor("xtb", [C, T], bf16, kind="ExternalInput")
    wqk_d = nc.dram_tensor("wqk", [C, 384], bf16, kind="ExternalInput")
    wv_d = nc.dram_tensor("wv", [C, 192], bf16, kind="ExternalInput")
    wp_d = nc.dram_tensor("wp", [256, C], bf16, kind="ExternalInput")
    out_d = nc.dram_tensor("out", [T, C], bf16, kind="ExternalOutput")

    with tile.TileContext(nc) as tc:
        with (
            tc.tile_pool(name="persist", bufs=1) as persist,
            tc.tile_pool(name="xpool", bufs=2) as xpool,
            tc.tile_pool(name="ps_st", bufs=2, space="PSUM") as ps_st,
            tc.tile_pool(name="ps_big", bufs=2, space="PSUM") as ps_big,
            tc.tile_pool(name="ps_yab", bufs=1, space="PSUM") as ps_yab,
            tc.tile_pool(name="etp", bufs=1) as etp,
            tc.tile_pool(name="small", bufs=4) as small,
            tc.tile_pool(name="outst", bufs=3) as outst,
        ):
            wqk = persist.tile([128, KC, 384], bf16, tag="wqk")
            wv = persist.tile([128, KC, 192], bf16, tag="wv")
            wp = persist.tile([128, 2, C], bf16, tag="wp")
            # per head: qk_h rows 0:64 = q (pre-scaled), 64:128 = k as
            # produced by the projection; k is then shifted to rows 0:64
            # of kt_h (SBUF->SBUF DMA) whose rows 64:128 are pre-zeroed so
            # score matmuls run K=128 (K=64 matmuls serialize LDWEIGHTS).
            qk = [
                persist.tile([128, T], bf16, tag=f"qk{h}", name=f"qk{h}")
                for h in range(3)
            ]
            kt = [
                persist.tile([128, T], bf16, tag=f"kt{h}", name=f"kt{h}")
                for h in range(3)
            ]
            # v in [token, dim] layout, 65 cols per head (65th col = ones
            # for the softmax denominator)
            v_all = persist.tile([128, 16, 195], bf16, tag="v")
            # y per token chunk: cols 0:192 = 3 heads x 64 dims; 192:256
            # zero so the second transpose window is full 128 cols
            y_all = persist.tile([128, 16, 256], bf16, tag="y")
            yt_all = persist.tile([128, 2, T], bf16, tag="yt")
            id_bf = persist.tile([128, 128], bf16, tag="idb")
            junk = persist.tile([128, 128], bf16, tag="junk")

            # ---- prologue: constants + persistent zero regions ----
            nc.vector.memset(junk[:, :], 0.0)
            for h in range(3):
                nc.vector.memset(kt[h][64:128, :], 0.0)
            v5 = v_all.rearrange("p t (h e) -> p t h e", e=65)
            nc.vector.memset(v5[:, :, :, 64:65], 1.0)
            nc.vector.memset(y_all[:, :, 192:256], 0.0)
            make_identity(nc, id_bf)

            # ---- weight loads: wqk ahead of xtb chunk 0 on the sync queue;
            # wv/wp (needed slightly later) ride the idle vector queue so
            # they don't delay the chunk-0 x stream ----
            for kc in range(KC):
                nc.gpsimd.dma_start(
                    out=wqk[:, kc, :], in_=wqk_d[128 * kc : 128 * kc + 128, :]
                )
            for kc in range(KC):
                nc.scalar.dma_start(
                    out=wv[:, kc, :], in_=wv_d[128 * kc : 128 * kc + 128, :]
                )
            nc.scalar.dma_start(out=wp[:, 0, :], in_=wp_d[0:128, :])
            nc.scalar.dma_start(out=wp[:, 1, :], in_=wp_d[128:256, :])

            # ---- PE warm-up on junk during the DMA prologue: keeps the
            # HAM clock ramping so qkv chunk 0 starts near 2.4 GHz ----
            for wi in range(14):
                pw = ps_big.tile([128, 512], f32, tag="big")
                nc.tensor.matmul(
                    pw[:, 0:128], lhsT=junk[:, :], rhs=junk[:, :],
                    start=True, stop=True,
                )

            xtiles = {}

            def emit_qkv_qk(c):
                ts = slice(512 * c, 512 * c + 512)
                xtb_c = xpool.tile([128, KC, 512], bf16, tag="xtb", name="xtb_c")
                xtiles[c] = xtb_c
                for kc in range(KC):
                    # chunk 0 splits across two queues to halve time-to-first-qkv
                    eng = nc.scalar if (c == 0 and kc >= 3) else nc.sync
                    eng.dma_start(
                        out=xtb_c[:, kc, :],
                        in_=xtb_d[128 * kc : 128 * kc + 128, ts],
                    )
                for m in range(3):
                    ps = ps_big.tile([128, 512], f32, tag="big")
                    for kc in range(KC):
                        nc.tensor.matmul(
                            ps[:, :],
                            lhsT=wqk[:, kc, 128 * m : 128 * m + 128],
                            rhs=xtb_c[:, kc, :],
                            start=(kc == 0),
                            stop=(kc == KC - 1),
                        )
                    nc.vector.tensor_copy(out=qk[m][:, ts], in_=ps[:, :])
                    # k shift on the gpsimd DMA queue: keeps the sync queue
                    # free for the input stream
                    nc.gpsimd.dma_start(out=kt[m][0:64, ts], in_=qk[m][64:128, ts])

            def emit_qkv_v(c):
                xtb_c = xtiles.pop(c)
                for tloc in range(4):
                    tcn = 4 * c + tloc
                    pv = ps_big.tile([128, 512], f32, tag="big")
                    for kc in range(KC):
                        nc.tensor.matmul(
                            pv[:, 0:192],
                            lhsT=xtb_c[:, kc, 128 * tloc : 128 * tloc + 128],
                            rhs=wv[:, kc, :],
                            start=(kc == 0),
                            stop=(kc == KC - 1),
                        )
                    nc.vector.tensor_copy(
                        out=v5[:, tcn, :, 0:64],
                        in_=pv[:, 0:192].rearrange("p (h e) -> p h e", e=64),
                    )

            def emit_scores_head(c, h, ets):
                i_base = 512 * c
                for p in range(2 * c + 2):
                    st = ps_st.tile([128, 1024], f32, tag="st")
                    et = etp.tile([128, 1024], bf16, tag=f"et{h}_{p}")
                    ets[(h, p)] = et
                    s0 = None
                    for half in range(2):
                        jc = 2 * p + half
                        i0 = 128 * (jc - 4 * c) if jc >= 4 * c else 0
                        if half == 0:
                            s0 = i0
                        lo = 512 * half
                        nc.tensor.matmul(
                            st[:, lo + i0 : lo + 512],
                            lhsT=kt[h][:, 128 * jc : 128 * jc + 128],
                            rhs=qk[h][:, i_base + i0 : i_base + 512],
                            start=True,
                            stop=True,
                        )
                    # one exp per pair tile; unwritten PSUM regions are
                    # only exp'd into et cols no y matmul ever reads
                    nc.scalar.activation(et[:, s0:1024], st[:, s0:1024], EXP)
                    for half in range(2):
                        jc = 2 * p + half
                        m = jc - 4 * c
                        if m >= 0:
                            # block-causal: queries of the lower half
                            # block can't see keys of the upper one
                            i0 = 512 * half + 128 * m
                            nc.vector.memset(et[64:128, i0 : i0 + 64], 0.0)

            def emit_y(c, ets):
                # y accumulation: r=0,1 in yA, r=2,3 in yB, head h at cols
                # 65h (+195 for odd r); col 65h+64 = softmax denominator.
                # Chains are kept atomic (consecutive matmuls) — interleaving
                # open PSUM accumulation chains with other matmuls breaks
                # the accumulation.
                yab = [
                    ps_yab.tile([128, 390], f32, tag="yA", name="yA"),
                    ps_yab.tile([128, 390], f32, tag="yB", name="yB"),
                ]
                # r-major: each r's 3 chains then its norm+proj, so the
                # stream-heavy proj matmuls interleave between LDWEIGHTS-heavy
                # y chains
                for r in range(4):
                    cr = 4 * c + r
                    yt_ = yab[r // 2]
                    base = 195 * (r & 1)
                    for h in range(3):
                        for jc in range(cr + 1):
                            nc.tensor.matmul(
                                yt_[:, base + 65 * h : base + 65 * h + 65],
                                lhsT=ets[(h, jc // 2)][
                                    :,
                                    512 * (jc & 1) + 128 * r : 512 * (jc & 1)
                                    + 128 * r
                                    + 128,
                                ],
                                rhs=v_all[:, jc, 65 * h : 65 * h + 65],
                                start=(jc == 0),
                                stop=(jc == cr),
                            )
                    emit_norm_proj(c, r, yab[r // 2])

            def emit_norm_proj(c, r, yt_):
                tcn = 4 * c + r
                tsl = slice(128 * tcn, 128 * tcn + 128)
                y3 = yt_.rearrange("p (g h e) -> p g h e", g=2, e=65)
                g = r & 1
                rec = small.tile([128, 3], f32, tag="rec")
                nc.vector.reciprocal(rec[:, :], y3[:, g, :, 64:65])
                for h in range(3):
                    nc.vector.tensor_scalar_mul(
                        y_all[:, tcn, 64 * h : 64 * h + 64],
                        y3[:, g, h, 0:64],
                        rec[:, h : h + 1],
                    )
                pt = ps_big.tile([128, 256], bf16, tag="big")
                nc.tensor.transpose(pt[:, 0:128], y_all[:, tcn, 0:128], id_bf)
                nc.tensor.transpose(pt[:, 128:256], y_all[:, tcn, 128:256], id_bf)
                nc.vector.tensor_copy(
                    out=yt_all[:, :, tsl],
                    in_=pt[:, :].rearrange("p (s t) -> p s t", s=2),
                )
                ot = outst.tile([128, C], bf16, tag="ot")
                for oc, ow in ((0, 512), (1, 256)):
                    pp = ps_big.tile([128, 512], f32, tag="big")
                    osl = slice(512 * oc, 512 * oc + ow)
                    nc.tensor.matmul(
                        pp[:, 0:ow],
                        lhsT=yt_all[:, 0, tsl],
                        rhs=wp[:, 0, osl],
                        start=True,
                        stop=False,
                    )
                    nc.tensor.matmul(
                        pp[:, 0:ow],
                        lhsT=yt_all[:, 1, tsl],
                        rhs=wp[:, 1, osl],
                        start=False,
                        stop=True,
                    )
                    # both copies on DVE: a waiting copy on the Scalar engine
                    # blocks ready EXPs behind it (engines are in-order)
                    nc.vector.tensor_copy(out=ot[:, osl], in_=pp[:, 0:ow])
                # out DMA on the gpsimd queue: its (late) data-ready wait must
                # not block the input loads streaming on the sync queue
                nc.gpsimd.dma_start(out=out_d[tsl, :], in_=ot[:, :])

            # software pipeline: qkv chunk c+1 is interleaved between the
            # score-head phases of group c, filling the PE while the Scalar
            # engine works through group c's exps
            emit_qkv_qk(0)
            emit_qkv_v(0)
            for c in range(4):
                ets = {}
                emit_scores_head(c, 0, ets)
                if c < 3:
                    emit_qkv_qk(c + 1)
                emit_scores_head(c, 1, ets)
                if c < 3:
                    emit_qkv_v(c + 1)
                emit_scores_head(c, 2, ets)
                emit_y(c, ets)

    nc.compile()
    return nc


def _get_nc():
    if "nc" not in _CACHE:
        _CACHE["nc"] = _build_bass()
    return _CACHE["nc"]


def make_in_maps(x, c_attn_w, c_proj_w, s):
    import ml_dtypes

    bf16 = ml_dtypes.bfloat16
    x = np.asarray(x, dtype=np.float32)
    c_attn_w = np.asarray(c_attn_w, dtype=np.float32)
    c_proj_w = np.asarray(c_proj_w, dtype=np.float32)
    s = np.asarray(s, dtype=np.float32)

    scale = np.float32(s[0] * np.log(T).astype(np.float32))
    f = np.float32(scale * np.float32(1.0 / np.sqrt(HD)))

    in_maps = []
    for b in range(2):
        xtb = np.ascontiguousarray(x[b].T).astype(bf16)  # [768, 2048]
        for g in range(4):
            hs = [3 * g, 3 * g + 1, 3 * g + 2]
            qrow = lambda h: c_attn_w[64 * h : 64 * h + 64] * f  # scaled q
            krow = lambda h: c_attn_w[C + 64 * h : C + 64 * h + 64]
            vrow = lambda h: c_attn_w[2 * C + 64 * h : 2 * C + 64 * h + 64]
            wqk = np.concatenate(
                [blk for h in hs for blk in (qrow(h), krow(h))], axis=0
            )  # [384, 768] rows = [q0,k0,q1,k1,q2,k2]
            wv = np.concatenate([vrow(h) for h in hs], axis=0)  # [192, 768]
            wp = np.zeros((256, C), np.float32)  # rows 192:256 stay zero
            wp[0:192] = c_proj_w[:, 192 * g : 192 * g + 192].T
            in_maps.append(
                {
                    "xtb": xtb,
                    "wqk": np.ascontiguousarray(wqk.T).astype(bf16),
                    "wv": np.ascontiguousarray(wv.T).astype(bf16),
                    "wp": wp.astype(bf16),
                }
            )
    return in_maps


def gather(results):
    out = np.empty((2, T, C), dtype=np.float32)
    for b in range(2):
        acc = results[4 * b]["out"].astype(np.float32)
        for g in range(1, 4):
            acc = acc + results[4 * b + g]["out"].astype(np.float32)
        out[b] = acc
    return out


def kernel(x, c_attn_w, c_proj_w, s):
    from concourse.bass_utils import run_bass_kernel_spmd

    nc = _get_nc()
    in_maps = make_in_maps(x, c_attn_w, c_proj_w, s)
    res = run_bass_kernel_spmd(nc, in_maps, list(range(N_CORES)))
    return gather(res.results)


# revision 35
# speedup vs baseline: 1.2968x; 1.2968x over previous
"""Block-causal self-attention (SSMax) Trainium2 kernel, v2 (pipelined).

Full inputs in, full output out. Sharding: 8 cores = 2 batches x 4 head
groups (3 heads each). Each core computes qkv for its head slice, the
block-causal attention for its 3 heads, and a partial c_proj product;
the host sums the 4 partials per batch.

v2 layout/schedule notes (per core):
  - x shipped twice: xt f32r [768,2048] feeds the q/k projection (f32r
    streams 1 cycle/row at >=256 moving cols, so no precision loss is
    paid for q/k), xtb bf16 feeds the v projection where lhsT=x chunks
    produce v directly in [token, dim] layout (no PE transposes).
  - Everything is software-pipelined per 512-token chunk c:
    qkv(c) -> scores/exp/y(c) -> proj(c), with attention group c
    consuming only k/v chunks <= c, so the Scalar-engine exp overlaps
    the whole run instead of serializing behind qkv.
  - Scores are computed transposed (ST[j,i] = k_j . q_i) in [128,1024]
    f32 PSUM pair-tiles (two 128-key chunks per tile); one exp per tile.
    q columns are pre-scaled by s*log(T)/sqrt(hd) on the host.
  - y accumulation chains (one per query chunk r x head) stay open
    across pair-tiles and are fed immediately after each pair's exp,
    keeping PE and ACT in lockstep. The softmax denominator comes from
    a ones-column appended to v (col 64 of each head's 65-col block).
  - Projection output is staged [0:512] via DVE and [0:256] via the
    Scalar engine to balance the two engines; partials ship bf16.
"""

import numpy as np

T = 2048
C = 768
HD = 64
KC = 6  # 768 / 128 contraction chunks
N_CORES = 8

_CACHE: dict = {}


def _build_bass():
    import concourse.bacc as bacc
    import concourse.mybir as mybir
    import concourse.tile as tile
    from concourse._compat import get_trn_type
    from concourse.masks import make_identity

    dt = mybir.dt
    f32 = dt.float32
    f32r = dt.float32r
    bf16 = dt.bfloat16
    EXP = mybir.ActivationFunctionType.Exp
    COPY = mybir.ActivationFunctionType.Copy

    nc = bacc.Bacc(get_trn_type() or "TRN2", debug=False)
    xtb_d = nc.dram_tensor("xtb", [C, T], bf16, kind="ExternalInput")
    wqk_d = nc.dram_tensor("wqk", [C, 384], bf16, kind="ExternalInput")
    wv_d = nc.dram_tensor("wv", [C, 192], bf16, kind="ExternalInput")
    wp_d = nc.dram_tensor("wp", [256, C], bf16, kind="ExternalInput")
    out_d = nc.dram_tensor("out", [T, C], bf16, kind="ExternalOutput")

    with tile.TileContext(nc) as tc:
        with (
            tc.tile_pool(name="persist", bufs=1) as persist,
            tc.tile_pool(name="xpool", bufs=2) as xpool,
            tc.tile_pool(name="ps_st", bufs=2, space="PSUM") as ps_st,
            tc.tile_pool(name="ps_big", bufs=2, space="PSUM") as ps_big,
            tc.tile_pool(name="ps_yab", bufs=1, space="PSUM") as ps_yab,
            tc.tile_pool(name="etp", bufs=1) as etp,
            tc.tile_pool(name="small", bufs=4) as small,
            tc.tile_pool(name="outst", bufs=3) as outst,
        ):
            wqk = persist.tile([128, KC, 384], bf16, tag="wqk")
            wv = persist.tile([128, KC, 192], bf16, tag="wv")
            wp = persist.tile([128, 2, C], bf16, tag="wp")
            # per head: qk_h rows 0:64 = q (pre-scaled), 64:128 = k as
            # produced by the projection; k is then shifted to rows 0:64
            # of kt_h (SBUF->SBUF DMA) whose rows 64:128 are pre-zeroed so
            # score matmuls run K=128 (K=64 matmuls serialize LDWEIGHTS).
            qk = [
                persist.tile([128, T], bf16, tag=f"qk{h}", name=f"qk{h}")
                for h in range(3)
            ]
            kt = [
                persist.tile([128, T], bf16, tag=f"kt{h}", name=f"kt{h}")
                for h in range(3)
            ]
            # v in [token, dim] layout, 65 cols per head (65th col = ones
            # for the softmax denominator)
            v_all = persist.tile([128, 16, 195], bf16, tag="v")
            # y per token chunk: cols 0:192 = 3 heads x 64 dims; 192:256
            # zero so the second transpose window is full 128 cols
            y_all = persist.tile([128, 16, 256], bf16, tag="y")
            yt_all = persist.tile([128, 2, T], bf16, tag="yt")
            id_bf = persist.tile([128, 128], bf16, tag="idb")
            junk = persist.tile([128, 128], bf16, tag="junk")

            # ---- prologue: constants + persistent zero regions ----
            nc.vector.memset(junk[:, :], 0.0)
            for h in range(3):
                nc.vector.memset(kt[h][64:128, :], 0.0)
            v5 = v_all.rearrange("p t (h e) -> p t h e", e=65)
            nc.vector.memset(v5[:, :, :, 64:65], 1.0)
            nc.vector.memset(y_all[:, :, 192:256], 0.0)
            make_identity(nc, id_bf)

            # ---- weight loads: wqk ahead of xtb chunk 0 on the sync queue;
            # wv/wp (needed slightly later) ride the idle vector queue so
            # they don't delay the chunk-0 x stream ----
            for kc in range(KC):
                nc.gpsimd.dma_start(
                    out=wqk[:, kc, :], in_=wqk_d[128 * kc : 128 * kc + 128, :]
                )
            for kc in range(KC):
                nc.scalar.dma_start(
                    out=wv[:, kc, :], in_=wv_d[128 * kc : 128 * kc + 128, :]
                )
            nc.scalar.dma_start(out=wp[:, 0, :], in_=wp_d[0:128, :])
            nc.scalar.dma_start(out=wp[:, 1, :], in_=wp_d[128:256, :])

            # ---- PE warm-up on junk during the DMA prologue: keeps the
            # HAM clock ramping so qkv chunk 0 starts near 2.4 GHz ----
            for wi in range(14):
                pw = ps_big.tile([128, 512], f32, tag="big")
                nc.tensor.matmul(
                    pw[:, 0:128], lhsT=junk[:, :], rhs=junk[:, :],
                    start=True, stop=True,
                )

            xtiles = {}

            def emit_qkv_qk(c):
                ts = slice(512 * c, 512 * c + 512)
                xtb_c = xpool.tile([128, KC, 512], bf16, tag="xtb", name="xtb_c")
                xtiles[c] = xtb_c
                for kc in range(KC):
                    # chunk 0 splits across two queues to halve time-to-first-qkv
                    eng = nc.scalar if (c == 0 and kc >= 3) else nc.sync
                    eng.dma_start(
                        out=xtb_c[:, kc, :],
                        in_=xtb_d[128 * kc : 128 * kc + 128, ts],
                    )
                for m in range(3):
                    ps = ps_big.tile([128, 512], f32, tag="big")
                    for kc in range(KC):
                        nc.tensor.matmul(
                            ps[:, :],
                            lhsT=wqk[:, kc, 128 * m : 128 * m + 128],
                            rhs=xtb_c[:, kc, :],
                            start=(kc == 0),
                            stop=(kc == KC - 1),
                        )
                    nc.vector.tensor_copy(out=qk[m][:, ts], in_=ps[:, :])
                    # k shift on the gpsimd DMA queue: keeps the sync queue
                    # free for the input stream
                    nc.gpsimd.dma_start(out=kt[m][0:64, ts], in_=qk[m][64:128, ts])

            def emit_qkv_v(c):
                xtb_c = xtiles.pop(c)
                for tloc in range(4):
                    tcn = 4 * c + tloc
                    pv = ps_big.tile([128, 512], f32, tag="big")
                    for kc in range(KC):
                        nc.tensor.matmul(
                            pv[:, 0:192],
                            lhsT=xtb_c[:, kc, 128 * tloc : 128 * tloc + 128],
                            rhs=wv[:, kc, :],
                            start=(kc == 0),
                            stop=(kc == KC - 1),
                        )
                    nc.vector.tensor_copy(
                        out=v5[:, tcn, :, 0:64],
                        in_=pv[:, 0:192].rearrange("p (h e) -> p h e", e=64),
                    )

            def emit_scores_head(c, h, ets):
                i_base = 512 * c
                for p in range(2 * c + 2):
                    st = ps_st.tile([128, 1024], f32, tag="st")
                    et = etp.tile([128, 1024], bf16, tag=f"et{h}_{p}")
                    ets[(h, p)] = et
                    s0 = None
                    for half in range(2):
                        jc = 2 * p + half
                        i0 = 128 * (jc - 4 * c) if jc >= 4 * c else 0
                        if half == 0:
                            s0 = i0
                        lo = 512 * half
                        nc.tensor.matmul(
                            st[:, lo + i0 : lo + 512],
                            lhsT=kt[h][:, 128 * jc : 128 * jc + 128],
                            rhs=qk[h][:, i_base + i0 : i_base + 512],
                            start=True,
                            stop=True,
                        )
                    # one exp per pair tile; unwritten PSUM regions are
                    # only exp'd into et cols no y matmul ever reads
                    nc.scalar.activation(et[:, s0:1024], st[:, s0:1024], EXP)
                    for half in range(2):
                        jc = 2 * p + half
                        m = jc - 4 * c
                        if m >= 0:
                            # block-causal: queries of the lower half
                            # block can't see keys of the upper one
                            i0 = 512 * half + 128 * m
                            nc.vector.memset(et[64:128, i0 : i0 + 64], 0.0)

            def emit_y(c, ets):
                # y accumulation: r=0,1 in yA, r=2,3 in yB, head h at cols
                # 65h (+195 for odd r); col 65h+64 = softmax denominator.
                # Chains are kept atomic (consecutive matmuls) — interleaving
                # open PSUM accumulation chains with other matmuls breaks
                # the accumulation.
                yab = [
                    ps_yab.tile([128, 390], f32, tag="yA", name="yA"),
                    ps_yab.tile([128, 390], f32, tag="yB", name="yB"),
                ]
                # all chains first (PE never waits on the DVE normalize
                # round-trip), then the four norm+proj sequences pipeline
                for h in range(3):
                    for r in range(4):
                        cr = 4 * c + r
                        yt_ = yab[r // 2]
                        base = 195 * (r & 1)
                        for jc in range(cr + 1):
                            nc.tensor.matmul(
                                yt_[:, base + 65 * h : base + 65 * h + 65],
                                lhsT=ets[(h, jc // 2)][
                                    :,
                                    512 * (jc & 1) + 128 * r : 512 * (jc & 1)
                                    + 128 * r
                                    + 128,
                                ],
                                rhs=v_all[:, jc, 65 * h : 65 * h + 65],
                                start=(jc == 0),
                                stop=(jc == cr),
                            )
                for r in range(4):
                    emit_norm_proj(c, r, yab[r // 2])

            def emit_norm_proj(c, r, yt_):
                tcn = 4 * c + r
                tsl = slice(128 * tcn, 128 * tcn + 128)
                y3 = yt_.rearrange("p (g h e) -> p g h e", g=2, e=65)
                g = r & 1
                rec = small.tile([128, 3], f32, tag="rec")
                nc.vector.reciprocal(rec[:, :], y3[:, g, :, 64:65])
                for h in range(3):
                    nc.vector.tensor_scalar_mul(
                        y_all[:, tcn, 64 * h : 64 * h + 64],
                        y3[:, g, h, 0:64],
                        rec[:, h : h + 1],
                    )
                pt = ps_big.tile([128, 256], bf16, tag="big")
                nc.tensor.transpose(pt[:, 0:128], y_all[:, tcn, 0:128], id_bf)
                nc.tensor.transpose(pt[:, 128:256], y_all[:, tcn, 128:256], id_bf)
                nc.vector.tensor_copy(
                    out=yt_all[:, :, tsl],
                    in_=pt[:, :].rearrange("p (s t) -> p s t", s=2),
                )
                ot = outst.tile([128, C], bf16, tag="ot")
                for oc, ow in ((0, 512), (1, 256)):
                    pp = ps_big.tile([128, 512], f32, tag="big")
                    osl = slice(512 * oc, 512 * oc + ow)
                    nc.tensor.matmul(
                        pp[:, 0:ow],
                        lhsT=yt_all[:, 0, tsl],
                        rhs=wp[:, 0, osl],
                        start=True,
                        stop=False,
                    )
                    nc.tensor.matmul(
                        pp[:, 0:ow],
                        lhsT=yt_all[:, 1, tsl],
                        rhs=wp[:, 1, osl],
                        start=False,
                        stop=True,
                    )
                    # both copies on DVE: a waiting copy on the Scalar engine
                    # blocks ready EXPs behind it (engines are in-order)
                    nc.vector.tensor_copy(out=ot[:, osl], in_=pp[:, 0:ow])
                # out DMA on the gpsimd queue: its (late) data-ready wait must
                # not block the input loads streaming on the sync queue
                nc.gpsimd.dma_start(out=out_d[tsl, :], in_=ot[:, :])

            # software pipeline: qkv chunk c+1 is interleaved between the
            # score-head phases of group c, filling the PE while the Scalar
            # engine works through group c's exps
            emit_qkv_qk(0)
            emit_qkv_v(0)
            for c in range(4):
                ets = {}
                emit_scores_head(c, 0, ets)
                if c < 3:
                    emit_qkv_qk(c + 1)
                emit_scores_head(c, 1, ets)
                if c < 3:
                    emit_qkv_v(c + 1)
                emit_scores_head(c, 2, ets)
                emit_y(c, ets)

    nc.compile()
    return nc


def _get_nc():
    if "nc" not in _CACHE:
        _CACHE["nc"] = _build_bass()
    return _CACHE["nc"]


def make_in_maps(x, c_attn_w, c_proj_w, s):
    import ml_dtypes

    bf16 = ml_dtypes.bfloat16
    x = np.asarray(x, dtype=np.float32)
    c_attn_w = np.asarray(c_attn_w, dtype=np.float32)
    c_proj_w = np.asarray(c_proj_w, dtype=np.float32)
    s = np.asarray(s, dtype=np.float32)

    scale = np.float32(s[0] * np.log(T).astype(np.float32))
    f = np.float32(scale * np.float32(1.0 / np.sqrt(HD)))

    in_maps = []
    for b in range(2):
        xtb = np.ascontiguousarray(x[b].T).astype(bf16)  # [768, 2048]
        for g in range(4):
            hs = [3 * g, 3 * g + 1, 3 * g + 2]
            qrow = lambda h: c_attn_w[64 * h : 64 * h + 64] * f  # scaled q
            krow = lambda h: c_attn_w[C + 64 * h : C + 64 * h + 64]
            vrow = lambda h: c_attn_w[2 * C + 64 * h : 2 * C + 64 * h + 64]
            wqk = np.concatenate(
                [blk for h in hs for blk in (qrow(h), krow(h))], axis=0
            )  # [384, 768] rows = [q0,k0,q1,k1,q2,k2]
            wv = np.concatenate([vrow(h) for h in hs], axis=0)  # [192, 768]
            wp = np.zeros((256, C), np.float32)  # rows 192:256 stay zero
            wp[0:192] = c_proj_w[:, 192 * g : 192 * g + 192].T
            in_maps.append(
                {
                    "xtb": xtb,
                    "wqk": np.ascontiguousarray(wqk.T).astype(bf16),
                    "wv": np.ascontiguousarray(wv.T).astype(bf16),
                    "wp": wp.astype(bf16),
                }
            )
    return in_maps


def gather(results):
    out = np.empty((2, T, C), dtype=np.float32)
    for b in range(2):
        acc = results[4 * b]["out"].astype(np.float32)
        for g in range(1, 4):
            acc = acc + results[4 * b + g]["out"].astype(np.float32)
        out[b] = acc
    return out


def kernel(x, c_attn_w, c_proj_w, s):
    from concourse.bass_utils import run_bass_kernel_spmd

    nc = _get_nc()
    in_maps = make_in_maps(x, c_attn_w, c_proj_w, s)
    res = run_bass_kernel_spmd(nc, in_maps, list(range(N_CORES)))
    return gather(res.results)


# revision 36
# speedup vs baseline: 1.3302x; 1.0258x over previous
"""Block-causal self-attention (SSMax) Trainium2 kernel, v2 (pipelined).

Full inputs in, full output out. Sharding: 8 cores = 2 batches x 4 head
groups (3 heads each). Each core computes qkv for its head slice, the
block-causal attention for its 3 heads, and a partial c_proj product;
the host sums the 4 partials per batch.

v2 layout/schedule notes (per core):
  - x shipped twice: xt f32r [768,2048] feeds the q/k projection (f32r
    streams 1 cycle/row at >=256 moving cols, so no precision loss is
    paid for q/k), xtb bf16 feeds the v projection where lhsT=x chunks
    produce v directly in [token, dim] layout (no PE transposes).
  - Everything is software-pipelined per 512-token chunk c:
    qkv(c) -> scores/exp/y(c) -> proj(c), with attention group c
    consuming only k/v chunks <= c, so the Scalar-engine exp overlaps
    the whole run instead of serializing behind qkv.
  - Scores are computed transposed (ST[j,i] = k_j . q_i) in [128,1024]
    f32 PSUM pair-tiles (two 128-key chunks per tile); one exp per tile.
    q columns are pre-scaled by s*log(T)/sqrt(hd) on the host.
  - y accumulation chains (one per query chunk r x head) stay open
    across pair-tiles and are fed immediately after each pair's exp,
    keeping PE and ACT in lockstep. The softmax denominator comes from
    a ones-column appended to v (col 64 of each head's 65-col block).
  - Projection output is staged [0:512] via DVE and [0:256] via the
    Scalar engine to balance the two engines; partials ship bf16.
"""

import numpy as np

T = 2048
C = 768
HD = 64
KC = 6  # 768 / 128 contraction chunks
N_CORES = 8

_CACHE: dict = {}


def _build_bass():
    import concourse.bacc as bacc
    import concourse.mybir as mybir
    import concourse.tile as tile
    from concourse._compat import get_trn_type
    from concourse.masks import make_identity

    dt = mybir.dt
    f32 = dt.float32
    f32r = dt.float32r
    bf16 = dt.bfloat16
    EXP = mybir.ActivationFunctionType.Exp
    COPY = mybir.ActivationFunctionType.Copy

    nc = bacc.Bacc(get_trn_type() or "TRN2", debug=False)
    xtb_d = nc.dram_tensor("xtb", [C, T], bf16, kind="ExternalInput")
    wqk_d = nc.dram_tensor("wqk", [C, 384], bf16, kind="ExternalInput")
    wv_d = nc.dram_tensor("wv", [C, 192], bf16, kind="ExternalInput")
    wp_d = nc.dram_tensor("wp", [256, C], bf16, kind="ExternalInput")
    out_d = nc.dram_tensor("out", [T, C], bf16, kind="ExternalOutput")

    with tile.TileContext(nc) as tc:
        with (
            tc.tile_pool(name="persist", bufs=1) as persist,
            tc.tile_pool(name="xpool", bufs=2) as xpool,
            tc.tile_pool(name="ps_st", bufs=2, space="PSUM") as ps_st,
            tc.tile_pool(name="ps_big", bufs=2, space="PSUM") as ps_big,
            tc.tile_pool(name="ps_yab", bufs=1, space="PSUM") as ps_yab,
            tc.tile_pool(name="etp", bufs=1) as etp,
            tc.tile_pool(name="small", bufs=4) as small,
            tc.tile_pool(name="outst", bufs=3) as outst,
        ):
            wqk = persist.tile([128, KC, 384], bf16, tag="wqk")
            wv = persist.tile([128, KC, 192], bf16, tag="wv")
            wp = persist.tile([128, 2, C], bf16, tag="wp")
            # per head: qk_h rows 0:64 = q (pre-scaled), 64:128 = k as
            # produced by the projection; k is then shifted to rows 0:64
            # of kt_h (SBUF->SBUF DMA) whose rows 64:128 are pre-zeroed so
            # score matmuls run K=128 (K=64 matmuls serialize LDWEIGHTS).
            qk = [
                persist.tile([128, T], bf16, tag=f"qk{h}", name=f"qk{h}")
                for h in range(3)
            ]
            kt = [
                persist.tile([128, T], bf16, tag=f"kt{h}", name=f"kt{h}")
                for h in range(3)
            ]
            # v in [token, dim] layout, 65 cols per head (65th col = ones
            # for the softmax denominator)
            v_all = persist.tile([128, 16, 195], bf16, tag="v")
            # y per token chunk: cols 0:192 = 3 heads x 64 dims; 192:256
            # zero so the second transpose window is full 128 cols
            y_all = persist.tile([128, 16, 256], bf16, tag="y")
            yt_all = persist.tile([128, 2, T], bf16, tag="yt")
            id_bf = persist.tile([128, 128], bf16, tag="idb")
            junk = persist.tile([128, 128], bf16, tag="junk")

            # ---- prologue: constants + persistent zero regions ----
            nc.vector.memset(junk[:, :], 0.0)
            for h in range(3):
                nc.vector.memset(kt[h][64:128, :], 0.0)
            v5 = v_all.rearrange("p t (h e) -> p t h e", e=65)
            nc.vector.memset(v5[:, :, :, 64:65], 1.0)
            nc.vector.memset(y_all[:, :, 192:256], 0.0)
            make_identity(nc, id_bf)

            # ---- weight loads: wqk ahead of xtb chunk 0 on the sync queue;
            # wv/wp (needed slightly later) ride the idle vector queue so
            # they don't delay the chunk-0 x stream ----
            for kc in range(KC):
                nc.gpsimd.dma_start(
                    out=wqk[:, kc, :], in_=wqk_d[128 * kc : 128 * kc + 128, :]
                )
            for kc in range(KC):
                nc.scalar.dma_start(
                    out=wv[:, kc, :], in_=wv_d[128 * kc : 128 * kc + 128, :]
                )
            nc.scalar.dma_start(out=wp[:, 0, :], in_=wp_d[0:128, :])
            nc.scalar.dma_start(out=wp[:, 1, :], in_=wp_d[128:256, :])

            # ---- PE warm-up on junk during the DMA prologue: keeps the
            # HAM clock ramping so qkv chunk 0 starts near 2.4 GHz ----
            for wi in range(14):
                pw = ps_big.tile([128, 512], f32, tag="big")
                nc.tensor.matmul(
                    pw[:, 0:128], lhsT=junk[:, :], rhs=junk[:, :],
                    start=True, stop=True,
                )

            xtiles = {}

            def emit_qkv_qk(c):
                ts = slice(512 * c, 512 * c + 512)
                xtb_c = xpool.tile([128, KC, 512], bf16, tag="xtb", name="xtb_c")
                xtiles[c] = xtb_c
                for kc in range(KC):
                    # chunk 0 splits across two queues to halve time-to-first-qkv
                    eng = nc.scalar if (c == 0 and kc >= 3) else nc.sync
                    eng.dma_start(
                        out=xtb_c[:, kc, :],
                        in_=xtb_d[128 * kc : 128 * kc + 128, ts],
                    )
                for m in range(3):
                    ps = ps_big.tile([128, 512], f32, tag="big")
                    for kc in range(KC):
                        nc.tensor.matmul(
                            ps[:, :],
                            lhsT=wqk[:, kc, 128 * m : 128 * m + 128],
                            rhs=xtb_c[:, kc, :],
                            start=(kc == 0),
                            stop=(kc == KC - 1),
                        )
                    nc.vector.tensor_copy(out=qk[m][:, ts], in_=ps[:, :])
                    # k shift on the gpsimd DMA queue: keeps the sync queue
                    # free for the input stream
                    nc.gpsimd.dma_start(out=kt[m][0:64, ts], in_=qk[m][64:128, ts])

            def emit_qkv_v(c):
                xtb_c = xtiles.pop(c)
                for tloc in range(4):
                    tcn = 4 * c + tloc
                    pv = ps_big.tile([128, 512], f32, tag="big")
                    for kc in range(KC):
                        nc.tensor.matmul(
                            pv[:, 0:192],
                            lhsT=xtb_c[:, kc, 128 * tloc : 128 * tloc + 128],
                            rhs=wv[:, kc, :],
                            start=(kc == 0),
                            stop=(kc == KC - 1),
                        )
                    nc.vector.tensor_copy(
                        out=v5[:, tcn, :, 0:64],
                        in_=pv[:, 0:192].rearrange("p (h e) -> p h e", e=64),
                    )

            def emit_scores_head(c, h, ets):
                i_base = 512 * c
                for p in range(2 * c + 2):
                    st = ps_st.tile([128, 1024], f32, tag="st")
                    # pairs 6/7 only exist for group 3 and are never live
                    # across two groups; single-buffer them to save SBUF
                    et = etp.tile(
                        [128, 1024],
                        bf16,
                        tag=f"et{h}_{p}",
                        name="et",
                        bufs=(2 if p < 6 else 1),
                    )
                    ets[(h, p)] = et
                    s0 = None
                    for half in range(2):
                        jc = 2 * p + half
                        i0 = 128 * (jc - 4 * c) if jc >= 4 * c else 0
                        if half == 0:
                            s0 = i0
                        lo = 512 * half
                        nc.tensor.matmul(
                            st[:, lo + i0 : lo + 512],
                            lhsT=kt[h][:, 128 * jc : 128 * jc + 128],
                            rhs=qk[h][:, i_base + i0 : i_base + 512],
                            start=True,
                            stop=True,
                        )
                    # one exp per pair tile; unwritten PSUM regions are
                    # only exp'd into et cols no y matmul ever reads
                    nc.scalar.activation(et[:, s0:1024], st[:, s0:1024], EXP)
                    for half in range(2):
                        jc = 2 * p + half
                        m = jc - 4 * c
                        if m >= 0:
                            # block-causal: queries of the lower half
                            # block can't see keys of the upper one
                            i0 = 512 * half + 128 * m
                            nc.vector.memset(et[64:128, i0 : i0 + 64], 0.0)

            def emit_y(c, ets):
                # y accumulation: r=0,1 in yA, r=2,3 in yB, head h at cols
                # 65h (+195 for odd r); col 65h+64 = softmax denominator.
                # Chains are kept atomic (consecutive matmuls) — interleaving
                # open PSUM accumulation chains with other matmuls breaks
                # the accumulation.
                yab = [
                    ps_yab.tile([128, 390], f32, tag="yA", name="yA"),
                    ps_yab.tile([128, 390], f32, tag="yB", name="yB"),
                ]
                # all chains first (PE never waits on the DVE normalize
                # round-trip), then the four norm+proj sequences pipeline
                for h in range(3):
                    for r in range(4):
                        cr = 4 * c + r
                        yt_ = yab[r // 2]
                        base = 195 * (r & 1)
                        for jc in range(cr + 1):
                            nc.tensor.matmul(
                                yt_[:, base + 65 * h : base + 65 * h + 65],
                                lhsT=ets[(h, jc // 2)][
                                    :,
                                    512 * (jc & 1) + 128 * r : 512 * (jc & 1)
                                    + 128 * r
                                    + 128,
                                ],
                                rhs=v_all[:, jc, 65 * h : 65 * h + 65],
                                start=(jc == 0),
                                stop=(jc == cr),
                            )
                for r in range(4):
                    emit_norm_proj(c, r, yab[r // 2])

            def emit_norm_proj(c, r, yt_):
                tcn = 4 * c + r
                tsl = slice(128 * tcn, 128 * tcn + 128)
                y3 = yt_.rearrange("p (g h e) -> p g h e", g=2, e=65)
                g = r & 1
                rec = small.tile([128, 3], f32, tag="rec")
                nc.vector.reciprocal(rec[:, :], y3[:, g, :, 64:65])
                for h in range(3):
                    nc.vector.tensor_scalar_mul(
                        y_all[:, tcn, 64 * h : 64 * h + 64],
                        y3[:, g, h, 0:64],
                        rec[:, h : h + 1],
                    )
                pt = ps_big.tile([128, 256], bf16, tag="big")
                nc.tensor.transpose(pt[:, 0:128], y_all[:, tcn, 0:128], id_bf)
                nc.tensor.transpose(pt[:, 128:256], y_all[:, tcn, 128:256], id_bf)
                nc.vector.tensor_copy(
                    out=yt_all[:, :, tsl],
                    in_=pt[:, :].rearrange("p (s t) -> p s t", s=2),
                )
                ot = outst.tile([128, C], bf16, tag="ot")
                for oc, ow in ((0, 512), (1, 256)):
                    pp = ps_big.tile([128, 512], f32, tag="big")
                    osl = slice(512 * oc, 512 * oc + ow)
                    nc.tensor.matmul(
                        pp[:, 0:ow],
                        lhsT=yt_all[:, 0, tsl],
                        rhs=wp[:, 0, osl],
                        start=True,
                        stop=False,
                    )
                    nc.tensor.matmul(
                        pp[:, 0:ow],
                        lhsT=yt_all[:, 1, tsl],
                        rhs=wp[:, 1, osl],
                        start=False,
                        stop=True,
                    )
                    # both copies on DVE: a waiting copy on the Scalar engine
                    # blocks ready EXPs behind it (engines are in-order)
                    nc.vector.tensor_copy(out=ot[:, osl], in_=pp[:, 0:ow])
                # out DMA on the gpsimd queue: its (late) data-ready wait must
                # not block the input loads streaming on the sync queue
                nc.gpsimd.dma_start(out=out_d[tsl, :], in_=ot[:, :])

            # software pipeline. Groups run in order 0,1,3,2 (the smaller
            # group-2 tail ends the kernel), qkv chunks are interleaved
            # between score-head phases, and each group's first score head is
            # prefetched before the previous group's y phase so the Scalar
            # engine never idles at group boundaries (et tiles double-buffer).
            ets_all = {c: {} for c in range(4)}
            emit_qkv_qk(0)
            emit_qkv_v(0)
            emit_scores_head(0, 0, ets_all[0])
            emit_qkv_qk(1)
            emit_scores_head(0, 1, ets_all[0])
            emit_qkv_v(1)
            emit_scores_head(0, 2, ets_all[0])
            order = [0, 1, 3, 2]
            for i, c in enumerate(order):
                nxt = order[i + 1] if i + 1 < 4 else None
                if nxt is not None:
                    emit_scores_head(nxt, 0, ets_all[nxt])
                emit_y(c, ets_all[c])
                if nxt is not None:
                    emit_scores_head(nxt, 1, ets_all[nxt])
                    if i == 0:
                        emit_qkv_qk(2)
                    emit_scores_head(nxt, 2, ets_all[nxt])
                    if i == 0:
                        emit_qkv_v(2)
                        emit_qkv_qk(3)
                        emit_qkv_v(3)

    nc.compile()
    return nc


def _get_nc():
    if "nc" not in _CACHE:
        _CACHE["nc"] = _build_bass()
    return _CACHE["nc"]


def make_in_maps(x, c_attn_w, c_proj_w, s):
    import ml_dtypes

    bf16 = ml_dtypes.bfloat16
    x = np.asarray(x, dtype=np.float32)
    c_attn_w = np.asarray(c_attn_w, dtype=np.float32)
    c_proj_w = np.asarray(c_proj_w, dtype=np.float32)
    s = np.asarray(s, dtype=np.float32)

    scale = np.float32(s[0] * np.log(T).astype(np.float32))
    f = np.float32(scale * np.float32(1.0 / np.sqrt(HD)))

    in_maps = []
    for b in range(2):
        xtb = np.ascontiguousarray(x[b].T).astype(bf16)  # [768, 2048]
        for g in range(4):
            hs = [3 * g, 3 * g + 1, 3 * g + 2]
            qrow = lambda h: c_attn_w[64 * h : 64 * h + 64] * f  # scaled q
            krow = lambda h: c_attn_w[C + 64 * h : C + 64 * h + 64]
            vrow = lambda h: c_attn_w[2 * C + 64 * h : 2 * C + 64 * h + 64]
            wqk = np.concatenate(
                [blk for h in hs for blk in (qrow(h), krow(h))], axis=0
            )  # [384, 768] rows = [q0,k0,q1,k1,q2,k2]
            wv = np.concatenate([vrow(h) for h in hs], axis=0)  # [192, 768]
            wp = np.zeros((256, C), np.float32)  # rows 192:256 stay zero
            wp[0:192] = c_proj_w[:, 192 * g : 192 * g + 192].T
            in_maps.append(
                {
                    "xtb": xtb,
                    "wqk": np.ascontiguousarray(wqk.T).astype(bf16),
                    "wv": np.ascontiguousarray(wv.T).astype(bf16),
                    "wp": wp.astype(bf16),
                }
            )
    return in_maps


def gather(results):
    out = np.empty((2, T, C), dtype=np.float32)
    for b in range(2):
        acc = results[4 * b]["out"].astype(np.float32)
        for g in range(1, 4):
            acc = acc + results[4 * b + g]["out"].astype(np.float32)
        out[b] = acc
    return out


def kernel(x, c_attn_w, c_proj_w, s):
    from concourse.bass_utils import run_bass_kernel_spmd

    nc = _get_nc()
    in_maps = make_in_maps(x, c_attn_w, c_proj_w, s)
    res = run_bass_kernel_spmd(nc, in_maps, list(range(N_CORES)))
    return gather(res.results)


# revision 37
# speedup vs baseline: 1.3551x; 1.0187x over previous
"""Block-causal self-attention (SSMax) Trainium2 kernel, v2 (pipelined).

Full inputs in, full output out. Sharding: 8 cores = 2 batches x 4 head
groups (3 heads each). Each core computes qkv for its head slice, the
block-causal attention for its 3 heads, and a partial c_proj product;
the host sums the 4 partials per batch.

v2 layout/schedule notes (per core):
  - x shipped twice: xt f32r [768,2048] feeds the q/k projection (f32r
    streams 1 cycle/row at >=256 moving cols, so no precision loss is
    paid for q/k), xtb bf16 feeds the v projection where lhsT=x chunks
    produce v directly in [token, dim] layout (no PE transposes).
  - Everything is software-pipelined per 512-token chunk c:
    qkv(c) -> scores/exp/y(c) -> proj(c), with attention group c
    consuming only k/v chunks <= c, so the Scalar-engine exp overlaps
    the whole run instead of serializing behind qkv.
  - Scores are computed transposed (ST[j,i] = k_j . q_i) in [128,1024]
    f32 PSUM pair-tiles (two 128-key chunks per tile); one exp per tile.
    q columns are pre-scaled by s*log(T)/sqrt(hd) on the host.
  - y accumulation chains (one per query chunk r x head) stay open
    across pair-tiles and are fed immediately after each pair's exp,
    keeping PE and ACT in lockstep. The softmax denominator comes from
    a ones-column appended to v (col 64 of each head's 65-col block).
  - Projection output is staged [0:512] via DVE and [0:256] via the
    Scalar engine to balance the two engines; partials ship bf16.
"""

import numpy as np

T = 2048
C = 768
HD = 64
KC = 6  # 768 / 128 contraction chunks
N_CORES = 8

_CACHE: dict = {}


def _build_bass():
    import concourse.bacc as bacc
    import concourse.mybir as mybir
    import concourse.tile as tile
    from concourse._compat import get_trn_type
    from concourse.masks import make_identity

    dt = mybir.dt
    f32 = dt.float32
    f32r = dt.float32r
    bf16 = dt.bfloat16
    EXP = mybir.ActivationFunctionType.Exp
    COPY = mybir.ActivationFunctionType.Copy

    nc = bacc.Bacc(get_trn_type() or "TRN2", debug=False)
    xtb_d = nc.dram_tensor("xtb", [C, T], bf16, kind="ExternalInput")
    wqk_d = nc.dram_tensor("wqk", [C, 384], bf16, kind="ExternalInput")
    wv_d = nc.dram_tensor("wv", [C, 192], bf16, kind="ExternalInput")
    wp_d = nc.dram_tensor("wp", [256, C], bf16, kind="ExternalInput")
    out_d = nc.dram_tensor("out", [T, C], bf16, kind="ExternalOutput")

    with tile.TileContext(nc) as tc:
        with (
            tc.tile_pool(name="persist", bufs=1) as persist,
            tc.tile_pool(name="xpool", bufs=2) as xpool,
            tc.tile_pool(name="ps_st", bufs=2, space="PSUM") as ps_st,
            tc.tile_pool(name="ps_big", bufs=2, space="PSUM") as ps_big,
            tc.tile_pool(name="ps_yab", bufs=1, space="PSUM") as ps_yab,
            tc.tile_pool(name="etp", bufs=1) as etp,
            tc.tile_pool(name="small", bufs=4) as small,
            tc.tile_pool(name="outst", bufs=3) as outst,
        ):
            wqk = persist.tile([128, KC, 384], bf16, tag="wqk")
            wv = persist.tile([128, KC, 192], bf16, tag="wv")
            wp = persist.tile([128, 2, C], bf16, tag="wp")
            # per head: qk_h rows 0:64 = q (pre-scaled), 64:128 = k as
            # produced by the projection; k is then shifted to rows 0:64
            # of kt_h (SBUF->SBUF DMA) whose rows 64:128 are pre-zeroed so
            # score matmuls run K=128 (K=64 matmuls serialize LDWEIGHTS).
            qk = [
                persist.tile([128, T], bf16, tag=f"qk{h}", name=f"qk{h}")
                for h in range(3)
            ]
            kt = [
                persist.tile([128, T], bf16, tag=f"kt{h}", name=f"kt{h}")
                for h in range(3)
            ]
            # v in [token, dim] layout, 65 cols per head (65th col = ones
            # for the softmax denominator)
            v_all = persist.tile([128, 16, 195], bf16, tag="v")
            # y per token chunk: cols 0:192 = 3 heads x 64 dims; 192:256
            # zero so the second transpose window is full 128 cols
            y_all = persist.tile([128, 16, 256], bf16, tag="y")
            yt_all = persist.tile([128, 2, T], bf16, tag="yt")
            id_bf = persist.tile([128, 128], bf16, tag="idb")
            junk = persist.tile([128, 128], bf16, tag="junk")

            # ---- prologue: constants + persistent zero regions ----
            nc.vector.memset(junk[:, :], 0.0)
            for h in range(3):
                nc.vector.memset(kt[h][64:128, :], 0.0)
            v5 = v_all.rearrange("p t (h e) -> p t h e", e=65)
            nc.vector.memset(v5[:, :, :, 64:65], 1.0)
            nc.vector.memset(y_all[:, :, 192:256], 0.0)
            make_identity(nc, id_bf)

            # ---- weight loads: wqk ahead of xtb chunk 0 on the sync queue;
            # wv/wp (needed slightly later) ride the idle vector queue so
            # they don't delay the chunk-0 x stream ----
            for kc in range(KC):
                nc.gpsimd.dma_start(
                    out=wqk[:, kc, :], in_=wqk_d[128 * kc : 128 * kc + 128, :]
                )
            for kc in range(KC):
                nc.scalar.dma_start(
                    out=wv[:, kc, :], in_=wv_d[128 * kc : 128 * kc + 128, :]
                )
            nc.scalar.dma_start(out=wp[:, 0, :], in_=wp_d[0:128, :])
            nc.scalar.dma_start(out=wp[:, 1, :], in_=wp_d[128:256, :])

            # ---- PE warm-up on junk during the DMA prologue: keeps the
            # HAM clock ramping so qkv chunk 0 starts near 2.4 GHz ----
            for wi in range(24):
                pw = ps_big.tile([128, 512], f32, tag="big")
                nc.tensor.matmul(
                    pw[:, 0:128], lhsT=junk[:, :], rhs=junk[:, :],
                    start=True, stop=True,
                )

            xtiles = {}

            def emit_qkv_qk(c):
                ts = slice(512 * c, 512 * c + 512)
                xtb_c = xpool.tile([128, KC, 512], bf16, tag="xtb", name="xtb_c")
                xtiles[c] = xtb_c
                for kc in range(KC):
                    # chunk 0 splits across two queues to halve time-to-first-qkv
                    eng = nc.scalar if (c == 0 and kc >= 3) else nc.sync
                    eng.dma_start(
                        out=xtb_c[:, kc, :],
                        in_=xtb_d[128 * kc : 128 * kc + 128, ts],
                    )
                for m in range(3):
                    ps = ps_big.tile([128, 512], f32, tag="big")
                    for kc in range(KC):
                        nc.tensor.matmul(
                            ps[:, :],
                            lhsT=wqk[:, kc, 128 * m : 128 * m + 128],
                            rhs=xtb_c[:, kc, :],
                            start=(kc == 0),
                            stop=(kc == KC - 1),
                        )
                    nc.vector.tensor_copy(out=qk[m][:, ts], in_=ps[:, :])
                    # k shift on the gpsimd DMA queue: keeps the sync queue
                    # free for the input stream
                    nc.gpsimd.dma_start(out=kt[m][0:64, ts], in_=qk[m][64:128, ts])

            def emit_qkv_v(c):
                xtb_c = xtiles.pop(c)
                for tloc in range(4):
                    tcn = 4 * c + tloc
                    pv = ps_big.tile([128, 512], f32, tag="big")
                    for kc in range(KC):
                        nc.tensor.matmul(
                            pv[:, 0:192],
                            lhsT=xtb_c[:, kc, 128 * tloc : 128 * tloc + 128],
                            rhs=wv[:, kc, :],
                            start=(kc == 0),
                            stop=(kc == KC - 1),
                        )
                    nc.vector.tensor_copy(
                        out=v5[:, tcn, :, 0:64],
                        in_=pv[:, 0:192].rearrange("p (h e) -> p h e", e=64),
                    )

            def emit_scores_head(c, h, ets):
                i_base = 512 * c
                for p in range(2 * c + 2):
                    st = ps_st.tile([128, 1024], f32, tag="st")
                    # pairs 6/7 only exist for group 3 and are never live
                    # across two groups; single-buffer them to save SBUF
                    et = etp.tile(
                        [128, 1024],
                        bf16,
                        tag=f"et{h}_{p}",
                        name="et",
                        bufs=(2 if p < 6 else 1),
                    )
                    ets[(h, p)] = et
                    s0 = None
                    for half in range(2):
                        jc = 2 * p + half
                        i0 = 128 * (jc - 4 * c) if jc >= 4 * c else 0
                        if half == 0:
                            s0 = i0
                        lo = 512 * half
                        nc.tensor.matmul(
                            st[:, lo + i0 : lo + 512],
                            lhsT=kt[h][:, 128 * jc : 128 * jc + 128],
                            rhs=qk[h][:, i_base + i0 : i_base + 512],
                            start=True,
                            stop=True,
                        )
                    # one exp per pair tile; unwritten PSUM regions are
                    # only exp'd into et cols no y matmul ever reads
                    nc.scalar.activation(et[:, s0:1024], st[:, s0:1024], EXP)
                    for half in range(2):
                        jc = 2 * p + half
                        m = jc - 4 * c
                        if m >= 0:
                            # block-causal: queries of the lower half
                            # block can't see keys of the upper one
                            i0 = 512 * half + 128 * m
                            nc.vector.memset(et[64:128, i0 : i0 + 64], 0.0)

            def emit_y(c, ets):
                # y accumulation: r=0,1 in yA, r=2,3 in yB, head h at cols
                # 65h (+195 for odd r); col 65h+64 = softmax denominator.
                # Chains are kept atomic (consecutive matmuls) — interleaving
                # open PSUM accumulation chains with other matmuls breaks
                # the accumulation.
                yab = [
                    ps_yab.tile([128, 390], f32, tag="yA", name="yA"),
                    ps_yab.tile([128, 390], f32, tag="yB", name="yB"),
                ]
                # all chains first (PE never waits on the DVE normalize
                # round-trip), then the four norm+proj sequences pipeline
                for h in range(3):
                    for r in range(4):
                        cr = 4 * c + r
                        yt_ = yab[r // 2]
                        base = 195 * (r & 1)
                        for jc in range(cr + 1):
                            nc.tensor.matmul(
                                yt_[:, base + 65 * h : base + 65 * h + 65],
                                lhsT=ets[(h, jc // 2)][
                                    :,
                                    512 * (jc & 1) + 128 * r : 512 * (jc & 1)
                                    + 128 * r
                                    + 128,
                                ],
                                rhs=v_all[:, jc, 65 * h : 65 * h + 65],
                                start=(jc == 0),
                                stop=(jc == cr),
                            )
                for r in range(4):
                    emit_norm_proj(c, r, yab[r // 2])

            def emit_norm_proj(c, r, yt_):
                tcn = 4 * c + r
                tsl = slice(128 * tcn, 128 * tcn + 128)
                y3 = yt_.rearrange("p (g h e) -> p g h e", g=2, e=65)
                g = r & 1
                rec = small.tile([128, 3], f32, tag="rec")
                nc.vector.reciprocal(rec[:, :], y3[:, g, :, 64:65])
                for h in range(3):
                    nc.vector.tensor_scalar_mul(
                        y_all[:, tcn, 64 * h : 64 * h + 64],
                        y3[:, g, h, 0:64],
                        rec[:, h : h + 1],
                    )
                pt = ps_big.tile([128, 256], bf16, tag="big")
                nc.tensor.transpose(pt[:, 0:128], y_all[:, tcn, 0:128], id_bf)
                nc.tensor.transpose(pt[:, 128:256], y_all[:, tcn, 128:256], id_bf)
                nc.vector.tensor_copy(
                    out=yt_all[:, :, tsl],
                    in_=pt[:, :].rearrange("p (s t) -> p s t", s=2),
                )
                ot = outst.tile([128, C], bf16, tag="ot")
                for oc, ow in ((0, 512), (1, 256)):
                    pp = ps_big.tile([128, 512], f32, tag="big")
                    osl = slice(512 * oc, 512 * oc + ow)
                    nc.tensor.matmul(
                        pp[:, 0:ow],
                        lhsT=yt_all[:, 0, tsl],
                        rhs=wp[:, 0, osl],
                        start=True,
                        stop=False,
                    )
                    nc.tensor.matmul(
                        pp[:, 0:ow],
                        lhsT=yt_all[:, 1, tsl],
                        rhs=wp[:, 1, osl],
                        start=False,
                        stop=True,
                    )
                    # both copies on DVE: a waiting copy on the Scalar engine
                    # blocks ready EXPs behind it (engines are in-order)
                    nc.vector.tensor_copy(out=ot[:, osl], in_=pp[:, 0:ow])
                # out DMA on the gpsimd queue: its (late) data-ready wait must
                # not block the input loads streaming on the sync queue
                nc.gpsimd.dma_start(out=out_d[tsl, :], in_=ot[:, :])

            # software pipeline. Groups run in order 0,1,3,2 (the smaller
            # group-2 tail ends the kernel), qkv chunks are interleaved
            # between score-head phases, and each group's first score head is
            # prefetched before the previous group's y phase so the Scalar
            # engine never idles at group boundaries (et tiles double-buffer).
            ets_all = {c: {} for c in range(4)}
            emit_qkv_qk(0)
            emit_qkv_v(0)
            emit_scores_head(0, 0, ets_all[0])
            emit_qkv_qk(1)
            emit_scores_head(0, 1, ets_all[0])
            emit_qkv_v(1)
            emit_scores_head(0, 2, ets_all[0])
            order = [0, 1, 3, 2]
            for i, c in enumerate(order):
                nxt = order[i + 1] if i + 1 < 4 else None
                if nxt is not None:
                    emit_scores_head(nxt, 0, ets_all[nxt])
                emit_y(c, ets_all[c])
                if nxt is not None:
                    emit_scores_head(nxt, 1, ets_all[nxt])
                    if i == 0:
                        emit_qkv_qk(2)
                    if i == 1:
                        emit_qkv_v(2)
                    emit_scores_head(nxt, 2, ets_all[nxt])
                    if i == 0:
                        emit_qkv_qk(3)
                    if i == 1:
                        emit_qkv_v(3)

    nc.compile()
    return nc


def _get_nc():
    if "nc" not in _CACHE:
        _CACHE["nc"] = _build_bass()
    return _CACHE["nc"]


def make_in_maps(x, c_attn_w, c_proj_w, s):
    import ml_dtypes

    bf16 = ml_dtypes.bfloat16
    x = np.asarray(x, dtype=np.float32)
    c_attn_w = np.asarray(c_attn_w, dtype=np.float32)
    c_proj_w = np.asarray(c_proj_w, dtype=np.float32)
    s = np.asarray(s, dtype=np.float32)

    scale = np.float32(s[0] * np.log(T).astype(np.float32))
    f = np.float32(scale * np.float32(1.0 / np.sqrt(HD)))

    in_maps = []
    for b in range(2):
        xtb = np.ascontiguousarray(x[b].T).astype(bf16)  # [768, 2048]
        for g in range(4):
            hs = [3 * g, 3 * g + 1, 3 * g + 2]
            qrow = lambda h: c_attn_w[64 * h : 64 * h + 64] * f  # scaled q
            krow = lambda h: c_attn_w[C + 64 * h : C + 64 * h + 64]
            vrow = lambda h: c_attn_w[2 * C + 64 * h : 2 * C + 64 * h + 64]
            wqk = np.concatenate(
                [blk for h in hs for blk in (qrow(h), krow(h))], axis=0
            )  # [384, 768] rows = [q0,k0,q1,k1,q2,k2]
            wv = np.concatenate([vrow(h) for h in hs], axis=0)  # [192, 768]
            wp = np.zeros((256, C), np.float32)  # rows 192:256 stay zero
            wp[0:192] = c_proj_w[:, 192 * g : 192 * g + 192].T
            in_maps.append(
                {
                    "xtb": xtb,
                    "wqk": np.ascontiguousarray(wqk.T).astype(bf16),
                    "wv": np.ascontiguousarray(wv.T).astype(bf16),
                    "wp": wp.astype(bf16),
                }
            )
    return in_maps


def gather(results):
    out = np.empty((2, T, C), dtype=np.float32)
    for b in range(2):
        acc = results[4 * b]["out"].astype(np.float32)
        for g in range(1, 4):
            acc = acc + results[4 * b + g]["out"].astype(np.float32)
        out[b] = acc
    return out


def kernel(x, c_attn_w, c_proj_w, s):
    from concourse.bass_utils import run_bass_kernel_spmd

    nc = _get_nc()
    in_maps = make_in_maps(x, c_attn_w, c_proj_w, s)
    res = run_bass_kernel_spmd(nc, in_maps, list(range(N_CORES)))
    return gather(res.results)


# revision 38
# speedup vs baseline: 1.3637x; 1.0063x over previous
"""Block-causal self-attention (SSMax) Trainium2 kernel, v2 (pipelined).

Full inputs in, full output out. Sharding: 8 cores = 2 batches x 4 head
groups (3 heads each). Each core computes qkv for its head slice, the
block-causal attention for its 3 heads, and a partial c_proj product;
the host sums the 4 partials per batch.

v2 layout/schedule notes (per core):
  - x shipped twice: xt f32r [768,2048] feeds the q/k projection (f32r
    streams 1 cycle/row at >=256 moving cols, so no precision loss is
    paid for q/k), xtb bf16 feeds the v projection where lhsT=x chunks
    produce v directly in [token, dim] layout (no PE transposes).
  - Everything is software-pipelined per 512-token chunk c:
    qkv(c) -> scores/exp/y(c) -> proj(c), with attention group c
    consuming only k/v chunks <= c, so the Scalar-engine exp overlaps
    the whole run instead of serializing behind qkv.
  - Scores are computed transposed (ST[j,i] = k_j . q_i) in [128,1024]
    f32 PSUM pair-tiles (two 128-key chunks per tile); one exp per tile.
    q columns are pre-scaled by s*log(T)/sqrt(hd) on the host.
  - y accumulation chains (one per query chunk r x head) stay open
    across pair-tiles and are fed immediately after each pair's exp,
    keeping PE and ACT in lockstep. The softmax denominator comes from
    a ones-column appended to v (col 64 of each head's 65-col block).
  - Projection output is staged [0:512] via DVE and [0:256] via the
    Scalar engine to balance the two engines; partials ship bf16.
"""

import numpy as np

T = 2048
C = 768
HD = 64
KC = 6  # 768 / 128 contraction chunks
N_CORES = 8

_CACHE: dict = {}


def _build_bass():
    import concourse.bacc as bacc
    import concourse.mybir as mybir
    import concourse.tile as tile
    from concourse._compat import get_trn_type
    from concourse.masks import make_identity

    dt = mybir.dt
    f32 = dt.float32
    f32r = dt.float32r
    bf16 = dt.bfloat16
    EXP = mybir.ActivationFunctionType.Exp
    COPY = mybir.ActivationFunctionType.Copy

    nc = bacc.Bacc(get_trn_type() or "TRN2", debug=False)
    xtb_d = nc.dram_tensor("xtb", [C, T], bf16, kind="ExternalInput")
    wqk_d = nc.dram_tensor("wqk", [C, 384], bf16, kind="ExternalInput")
    wv_d = nc.dram_tensor("wv", [C, 192], bf16, kind="ExternalInput")
    wp_d = nc.dram_tensor("wp", [256, C], bf16, kind="ExternalInput")
    out_d = nc.dram_tensor("out", [T, C], bf16, kind="ExternalOutput")

    with tile.TileContext(nc) as tc:
        with (
            tc.tile_pool(name="persist", bufs=1) as persist,
            tc.tile_pool(name="xpool", bufs=2) as xpool,
            tc.tile_pool(name="ps_st", bufs=2, space="PSUM") as ps_st,
            tc.tile_pool(name="ps_big", bufs=2, space="PSUM") as ps_big,
            tc.tile_pool(name="ps_yab", bufs=1, space="PSUM") as ps_yab,
            tc.tile_pool(name="etp", bufs=1) as etp,
            tc.tile_pool(name="small", bufs=4) as small,
            tc.tile_pool(name="outst", bufs=3) as outst,
        ):
            wqk = persist.tile([128, KC, 384], bf16, tag="wqk")
            wv = persist.tile([128, KC, 192], bf16, tag="wv")
            wp = persist.tile([128, 2, C], bf16, tag="wp")
            # per head: qk_h rows 0:64 = q (pre-scaled), 64:128 = k as
            # produced by the projection; k is then shifted to rows 0:64
            # of kt_h (SBUF->SBUF DMA) whose rows 64:128 are pre-zeroed so
            # score matmuls run K=128 (K=64 matmuls serialize LDWEIGHTS).
            qk = [
                persist.tile([128, T], bf16, tag=f"qk{h}", name=f"qk{h}")
                for h in range(3)
            ]
            kt = [
                persist.tile([128, T], bf16, tag=f"kt{h}", name=f"kt{h}")
                for h in range(3)
            ]
            # v in [token, dim] layout, 65 cols per head (65th col = ones
            # for the softmax denominator)
            v_all = persist.tile([128, 16, 195], bf16, tag="v")
            # y per token chunk: cols 0:192 = 3 heads x 64 dims; 192:256
            # zero so the second transpose window is full 128 cols
            y_all = persist.tile([128, 16, 256], bf16, tag="y")
            yt_all = persist.tile([128, 2, T], bf16, tag="yt")
            id_bf = persist.tile([128, 128], bf16, tag="idb")
            junk = persist.tile([128, 128], bf16, tag="junk")

            # ---- prologue: constants + persistent zero regions ----
            nc.vector.memset(junk[:, :], 0.0)
            for h in range(3):
                nc.vector.memset(kt[h][64:128, :], 0.0)
            v5 = v_all.rearrange("p t (h e) -> p t h e", e=65)
            nc.vector.memset(v5[:, :, :, 64:65], 1.0)
            nc.vector.memset(y_all[:, :, 192:256], 0.0)
            make_identity(nc, id_bf)

            # ---- weight loads: wqk ahead of xtb chunk 0 on the sync queue;
            # wv/wp (needed slightly later) ride the idle vector queue so
            # they don't delay the chunk-0 x stream ----
            for kc in range(KC):
                nc.gpsimd.dma_start(
                    out=wqk[:, kc, :], in_=wqk_d[128 * kc : 128 * kc + 128, :]
                )
            for kc in range(KC):
                nc.scalar.dma_start(
                    out=wv[:, kc, :], in_=wv_d[128 * kc : 128 * kc + 128, :]
                )
            nc.scalar.dma_start(out=wp[:, 0, :], in_=wp_d[0:128, :])
            nc.scalar.dma_start(out=wp[:, 1, :], in_=wp_d[128:256, :])

            # ---- PE warm-up on junk during the DMA prologue: keeps the
            # HAM clock ramping so qkv chunk 0 starts near 2.4 GHz ----
            for wi in range(24):
                pw = ps_big.tile([128, 512], f32, tag="big")
                nc.tensor.matmul(
                    pw[:, 0:128], lhsT=junk[:, :], rhs=junk[:, :],
                    start=True, stop=True,
                )

            xtiles = {}

            def emit_qkv_m(c, m):
                ts = slice(512 * c, 512 * c + 512)
                if m == 0:
                    xtb_c = xpool.tile(
                        [128, KC, 512], bf16, tag="xtb", name="xtb_c"
                    )
                    xtiles[c] = xtb_c
                    for kc in range(KC):
                        # chunk 0 splits across two queues to halve
                        # time-to-first-qkv
                        eng = nc.scalar if (c == 0 and kc >= 3) else nc.sync
                        eng.dma_start(
                            out=xtb_c[:, kc, :],
                            in_=xtb_d[128 * kc : 128 * kc + 128, ts],
                        )
                xtb_c = xtiles[c]
                ps = ps_big.tile([128, 512], f32, tag="big")
                for kc in range(KC):
                    nc.tensor.matmul(
                        ps[:, :],
                        lhsT=wqk[:, kc, 128 * m : 128 * m + 128],
                        rhs=xtb_c[:, kc, :],
                        start=(kc == 0),
                        stop=(kc == KC - 1),
                    )
                nc.vector.tensor_copy(out=qk[m][:, ts], in_=ps[:, :])
                # k shift on the gpsimd DMA queue: keeps the sync queue
                # free for the input stream
                nc.gpsimd.dma_start(out=kt[m][0:64, ts], in_=qk[m][64:128, ts])

            def emit_qkv_qk(c):
                for m in range(3):
                    emit_qkv_m(c, m)

            def emit_qkv_v(c):
                xtb_c = xtiles.pop(c)
                for tloc in range(4):
                    tcn = 4 * c + tloc
                    pv = ps_big.tile([128, 512], f32, tag="big")
                    for kc in range(KC):
                        nc.tensor.matmul(
                            pv[:, 0:192],
                            lhsT=xtb_c[:, kc, 128 * tloc : 128 * tloc + 128],
                            rhs=wv[:, kc, :],
                            start=(kc == 0),
                            stop=(kc == KC - 1),
                        )
                    nc.vector.tensor_copy(
                        out=v5[:, tcn, :, 0:64],
                        in_=pv[:, 0:192].rearrange("p (h e) -> p h e", e=64),
                    )

            def emit_scores_head(c, h, ets):
                i_base = 512 * c
                for p in range(2 * c + 2):
                    st = ps_st.tile([128, 1024], f32, tag="st")
                    # pairs 6/7 only exist for group 3 and are never live
                    # across two groups; single-buffer them to save SBUF
                    et = etp.tile(
                        [128, 1024],
                        bf16,
                        tag=f"et{h}_{p}",
                        name="et",
                        bufs=(2 if p < 6 else 1),
                    )
                    ets[(h, p)] = et
                    s0 = None
                    for half in range(2):
                        jc = 2 * p + half
                        i0 = 128 * (jc - 4 * c) if jc >= 4 * c else 0
                        if half == 0:
                            s0 = i0
                        lo = 512 * half
                        nc.tensor.matmul(
                            st[:, lo + i0 : lo + 512],
                            lhsT=kt[h][:, 128 * jc : 128 * jc + 128],
                            rhs=qk[h][:, i_base + i0 : i_base + 512],
                            start=True,
                            stop=True,
                        )
                    # one exp per pair tile; unwritten PSUM regions are
                    # only exp'd into et cols no y matmul ever reads
                    nc.scalar.activation(et[:, s0:1024], st[:, s0:1024], EXP)
                    for half in range(2):
                        jc = 2 * p + half
                        m = jc - 4 * c
                        if m >= 0:
                            # block-causal: queries of the lower half
                            # block can't see keys of the upper one
                            i0 = 512 * half + 128 * m
                            nc.vector.memset(et[64:128, i0 : i0 + 64], 0.0)

            def emit_y(c, ets):
                # y accumulation: r=0,1 in yA, r=2,3 in yB, head h at cols
                # 65h (+195 for odd r); col 65h+64 = softmax denominator.
                # Chains are kept atomic (consecutive matmuls) — interleaving
                # open PSUM accumulation chains with other matmuls breaks
                # the accumulation.
                yab = [
                    ps_yab.tile([128, 390], f32, tag="yA", name="yA"),
                    ps_yab.tile([128, 390], f32, tag="yB", name="yB"),
                ]
                # all chains first (PE never waits on the DVE normalize
                # round-trip), then the four norm+proj sequences pipeline
                for h in range(3):
                    for r in range(4):
                        cr = 4 * c + r
                        yt_ = yab[r // 2]
                        base = 195 * (r & 1)
                        for jc in range(cr + 1):
                            nc.tensor.matmul(
                                yt_[:, base + 65 * h : base + 65 * h + 65],
                                lhsT=ets[(h, jc // 2)][
                                    :,
                                    512 * (jc & 1) + 128 * r : 512 * (jc & 1)
                                    + 128 * r
                                    + 128,
                                ],
                                rhs=v_all[:, jc, 65 * h : 65 * h + 65],
                                start=(jc == 0),
                                stop=(jc == cr),
                            )
                for r in range(4):
                    emit_norm_proj(c, r, yab[r // 2])

            def emit_norm_proj(c, r, yt_):
                tcn = 4 * c + r
                tsl = slice(128 * tcn, 128 * tcn + 128)
                y3 = yt_.rearrange("p (g h e) -> p g h e", g=2, e=65)
                g = r & 1
                rec = small.tile([128, 3], f32, tag="rec")
                nc.vector.reciprocal(rec[:, :], y3[:, g, :, 64:65])
                for h in range(3):
                    nc.vector.tensor_scalar_mul(
                        y_all[:, tcn, 64 * h : 64 * h + 64],
                        y3[:, g, h, 0:64],
                        rec[:, h : h + 1],
                    )
                pt = ps_big.tile([128, 256], bf16, tag="big")
                nc.tensor.transpose(pt[:, 0:128], y_all[:, tcn, 0:128], id_bf)
                nc.tensor.transpose(pt[:, 128:256], y_all[:, tcn, 128:256], id_bf)
                nc.vector.tensor_copy(
                    out=yt_all[:, :, tsl],
                    in_=pt[:, :].rearrange("p (s t) -> p s t", s=2),
                )
                ot = outst.tile([128, C], bf16, tag="ot")
                for oc, ow in ((0, 512), (1, 256)):
                    pp = ps_big.tile([128, 512], f32, tag="big")
                    osl = slice(512 * oc, 512 * oc + ow)
                    nc.tensor.matmul(
                        pp[:, 0:ow],
                        lhsT=yt_all[:, 0, tsl],
                        rhs=wp[:, 0, osl],
                        start=True,
                        stop=False,
                    )
                    nc.tensor.matmul(
                        pp[:, 0:ow],
                        lhsT=yt_all[:, 1, tsl],
                        rhs=wp[:, 1, osl],
                        start=False,
                        stop=True,
                    )
                    # both copies on DVE: a waiting copy on the Scalar engine
                    # blocks ready EXPs behind it (engines are in-order)
                    nc.vector.tensor_copy(out=ot[:, osl], in_=pp[:, 0:ow])
                # out DMA on the gpsimd queue: its (late) data-ready wait must
                # not block the input loads streaming on the sync queue
                nc.gpsimd.dma_start(out=out_d[tsl, :], in_=ot[:, :])

            # software pipeline. Groups run in order 0,1,3,2 (the smaller
            # group-2 tail ends the kernel), qkv chunks are interleaved
            # between score-head phases, and each group's first score head is
            # prefetched before the previous group's y phase so the Scalar
            # engine never idles at group boundaries (et tiles double-buffer).
            ets_all = {c: {} for c in range(4)}
            # chunk-0 front: head h's scores need only qkv m-chunk h, so the
            # first exps fire right after m0 instead of after the full chunk
            emit_qkv_m(0, 0)
            emit_scores_head(0, 0, ets_all[0])
            emit_qkv_m(0, 1)
            emit_scores_head(0, 1, ets_all[0])
            emit_qkv_m(0, 2)
            emit_qkv_v(0)
            emit_scores_head(0, 2, ets_all[0])
            emit_qkv_qk(1)
            emit_qkv_v(1)
            order = [0, 1, 3, 2]
            for i, c in enumerate(order):
                nxt = order[i + 1] if i + 1 < 4 else None
                if nxt is not None:
                    emit_scores_head(nxt, 0, ets_all[nxt])
                emit_y(c, ets_all[c])
                if nxt is not None:
                    emit_scores_head(nxt, 1, ets_all[nxt])
                    if i == 0:
                        emit_qkv_qk(2)
                    if i == 1:
                        emit_qkv_v(2)
                    emit_scores_head(nxt, 2, ets_all[nxt])
                    if i == 0:
                        emit_qkv_qk(3)
                    if i == 1:
                        emit_qkv_v(3)

    nc.compile()
    return nc


def _get_nc():
    if "nc" not in _CACHE:
        _CACHE["nc"] = _build_bass()
    return _CACHE["nc"]


def make_in_maps(x, c_attn_w, c_proj_w, s):
    import ml_dtypes

    bf16 = ml_dtypes.bfloat16
    x = np.asarray(x, dtype=np.float32)
    c_attn_w = np.asarray(c_attn_w, dtype=np.float32)
    c_proj_w = np.asarray(c_proj_w, dtype=np.float32)
    s = np.asarray(s, dtype=np.float32)

    scale = np.float32(s[0] * np.log(T).astype(np.float32))
    f = np.float32(scale * np.float32(1.0 / np.sqrt(HD)))

    in_maps = []
    for b in range(2):
        xtb = np.ascontiguousarray(x[b].T).astype(bf16)  # [768, 2048]
        for g in range(4):
            hs = [3 * g, 3 * g + 1, 3 * g + 2]
            qrow = lambda h: c_attn_w[64 * h : 64 * h + 64] * f  # scaled q
            krow = lambda h: c_attn_w[C + 64 * h : C + 64 * h + 64]
            vrow = lambda h: c_attn_w[2 * C + 64 * h : 2 * C + 64 * h + 64]
            wqk = np.concatenate(
                [blk for h in hs for blk in (qrow(h), krow(h))], axis=0
            )  # [384, 768] rows = [q0,k0,q1,k1,q2,k2]
            wv = np.concatenate([vrow(h) for h in hs], axis=0)  # [192, 768]
            wp = np.zeros((256, C), np.float32)  # rows 192:256 stay zero
            wp[0:192] = c_proj_w[:, 192 * g : 192 * g + 192].T
            in_maps.append(
                {
                    "xtb": xtb,
                    "wqk": np.ascontiguousarray(wqk.T).astype(bf16),
                    "wv": np.ascontiguousarray(wv.T).astype(bf16),
                    "wp": wp.astype(bf16),
                }
            )
    return in_maps


def gather(results):
    out = np.empty((2, T, C), dtype=np.float32)
    for b in range(2):
        acc = results[4 * b]["out"].astype(np.float32)
        for g in range(1, 4):
            acc = acc + results[4 * b + g]["out"].astype(np.float32)
        out[b] = acc
    return out


def kernel(x, c_attn_w, c_proj_w, s):
    from concourse.bass_utils import run_bass_kernel_spmd

    nc = _get_nc()
    in_maps = make_in_maps(x, c_attn_w, c_proj_w, s)
    res = run_bass_kernel_spmd(nc, in_maps, list(range(N_CORES)))
    return gather(res.results)


# revision 39
# speedup vs baseline: 1.3946x; 1.0227x over previous
"""Block-causal self-attention (SSMax) Trainium2 kernel, v2 (pipelined).

Full inputs in, full output out. Sharding: 8 cores = 2 batches x 4 head
groups (3 heads each). Each core computes qkv for its head slice, the
block-causal attention for its 3 heads, and a partial c_proj product;
the host sums the 4 partials per batch.

v2 layout/schedule notes (per core):
  - x shipped twice: xt f32r [768,2048] feeds the q/k projection (f32r
    streams 1 cycle/row at >=256 moving cols, so no precision loss is
    paid for q/k), xtb bf16 feeds the v projection where lhsT=x chunks
    produce v directly in [token, dim] layout (no PE transposes).
  - Everything is software-pipelined per 512-token chunk c:
    qkv(c) -> scores/exp/y(c) -> proj(c), with attention group c
    consuming only k/v chunks <= c, so the Scalar-engine exp overlaps
    the whole run instead of serializing behind qkv.
  - Scores are computed transposed (ST[j,i] = k_j . q_i) in [128,1024]
    f32 PSUM pair-tiles (two 128-key chunks per tile); one exp per tile.
    q columns are pre-scaled by s*log(T)/sqrt(hd) on the host.
  - y accumulation chains (one per query chunk r x head) stay open
    across pair-tiles and are fed immediately after each pair's exp,
    keeping PE and ACT in lockstep. The softmax denominator comes from
    a ones-column appended to v (col 64 of each head's 65-col block).
  - Projection output is staged [0:512] via DVE and [0:256] via the
    Scalar engine to balance the two engines; partials ship bf16.
"""

import numpy as np

T = 2048
C = 768
HD = 64
KC = 6  # 768 / 128 contraction chunks
N_CORES = 8

_CACHE: dict = {}


def _build_bass():
    import concourse.bacc as bacc
    import concourse.mybir as mybir
    import concourse.tile as tile
    from concourse._compat import get_trn_type
    from concourse.masks import make_identity

    dt = mybir.dt
    f32 = dt.float32
    f32r = dt.float32r
    bf16 = dt.bfloat16
    EXP = mybir.ActivationFunctionType.Exp
    COPY = mybir.ActivationFunctionType.Copy

    nc = bacc.Bacc(get_trn_type() or "TRN2", debug=False)
    xtb_d = nc.dram_tensor("xtb", [C, T], bf16, kind="ExternalInput")
    wqk_d = nc.dram_tensor("wqk", [C, 384], bf16, kind="ExternalInput")
    wv_d = nc.dram_tensor("wv", [C, 192], bf16, kind="ExternalInput")
    wp_d = nc.dram_tensor("wp", [256, C], bf16, kind="ExternalInput")
    out_d = nc.dram_tensor("out", [T, C], bf16, kind="ExternalOutput")

    with tile.TileContext(nc) as tc:
        with (
            tc.tile_pool(name="persist", bufs=1) as persist,
            tc.tile_pool(name="xpool", bufs=2) as xpool,
            tc.tile_pool(name="ps_st", bufs=2, space="PSUM") as ps_st,
            tc.tile_pool(name="ps_big", bufs=2, space="PSUM") as ps_big,
            tc.tile_pool(name="ps_yab", bufs=1, space="PSUM") as ps_yab,
            tc.tile_pool(name="etp", bufs=1) as etp,
            tc.tile_pool(name="small", bufs=4) as small,
            tc.tile_pool(name="outst", bufs=3) as outst,
        ):
            wqk = persist.tile([128, KC, 384], bf16, tag="wqk")
            wv = persist.tile([128, KC, 192], bf16, tag="wv")
            wp = persist.tile([128, 2, C], bf16, tag="wp")
            # per head: qk_h rows 0:64 = q (pre-scaled), 64:128 = k as
            # produced by the projection; k is then shifted to rows 0:64
            # of kt_h (SBUF->SBUF DMA) whose rows 64:128 are pre-zeroed so
            # score matmuls run K=128 (K=64 matmuls serialize LDWEIGHTS).
            qk = [
                persist.tile([128, T], bf16, tag=f"qk{h}", name=f"qk{h}")
                for h in range(3)
            ]
            kt = [
                persist.tile([128, T], bf16, tag=f"kt{h}", name=f"kt{h}")
                for h in range(3)
            ]
            # v in [token, dim] layout, 65 cols per head (65th col = ones
            # for the softmax denominator)
            v_all = persist.tile([128, 16, 195], bf16, tag="v")
            # y per token chunk: cols 0:192 = 3 heads x 64 dims; 192:256
            # zero so the second transpose window is full 128 cols
            y_all = persist.tile([128, 16, 256], bf16, tag="y")
            yt_all = persist.tile([128, 2, T], bf16, tag="yt")
            id_bf = persist.tile([128, 128], bf16, tag="idb")
            junk = persist.tile([128, 128], bf16, tag="junk")

            # ---- prologue: constants + persistent zero regions ----
            nc.vector.memset(junk[:, :], 0.0)
            for h in range(3):
                nc.vector.memset(kt[h][64:128, :], 0.0)
            v5 = v_all.rearrange("p t (h e) -> p t h e", e=65)
            nc.vector.memset(v5[:, :, :, 64:65], 1.0)
            nc.vector.memset(y_all[:, :, 192:256], 0.0)
            make_identity(nc, id_bf)

            # ---- weight loads: wqk ahead of xtb chunk 0 on the sync queue;
            # wv/wp (needed slightly later) ride the idle vector queue so
            # they don't delay the chunk-0 x stream ----
            for kc in range(KC):
                nc.gpsimd.dma_start(
                    out=wqk[:, kc, :], in_=wqk_d[128 * kc : 128 * kc + 128, :]
                )
            # wp alone rides the scalar queue (tiny, keeps the ACT table
            # load early); wv go on sync behind the chunk-0 x stream
            nc.scalar.dma_start(out=wp[:, 0, :], in_=wp_d[0:128, :])
            nc.scalar.dma_start(out=wp[:, 1, :], in_=wp_d[128:256, :])

            # ---- PE warm-up on junk during the DMA prologue: keeps the
            # HAM clock ramping so qkv chunk 0 starts near 2.4 GHz ----
            for wi in range(24):
                pw = ps_big.tile([128, 512], f32, tag="big")
                nc.tensor.matmul(
                    pw[:, 0:128], lhsT=junk[:, :], rhs=junk[:, :],
                    start=True, stop=True,
                )

            xtiles = {}

            def emit_qkv_m(c, m):
                ts = slice(512 * c, 512 * c + 512)
                if m == 0:
                    xtb_c = xpool.tile(
                        [128, KC, 512], bf16, tag="xtb", name="xtb_c"
                    )
                    xtiles[c] = xtb_c
                    for kc in range(KC):
                        nc.sync.dma_start(
                            out=xtb_c[:, kc, :],
                            in_=xtb_d[128 * kc : 128 * kc + 128, ts],
                        )
                    if c == 0:
                        # wv after the chunk-0 stream, before chunk 1
                        for kc in range(KC):
                            nc.sync.dma_start(
                                out=wv[:, kc, :],
                                in_=wv_d[128 * kc : 128 * kc + 128, :],
                            )
                xtb_c = xtiles[c]
                ps = ps_big.tile([128, 512], f32, tag="big")
                for kc in range(KC):
                    nc.tensor.matmul(
                        ps[:, :],
                        lhsT=wqk[:, kc, 128 * m : 128 * m + 128],
                        rhs=xtb_c[:, kc, :],
                        start=(kc == 0),
                        stop=(kc == KC - 1),
                    )
                nc.vector.tensor_copy(out=qk[m][:, ts], in_=ps[:, :])
                # k shift on the gpsimd DMA queue: keeps the sync queue
                # free for the input stream
                nc.gpsimd.dma_start(out=kt[m][0:64, ts], in_=qk[m][64:128, ts])

            def emit_qkv_qk(c):
                for m in range(3):
                    emit_qkv_m(c, m)

            def emit_qkv_v(c):
                xtb_c = xtiles.pop(c)
                for tloc in range(4):
                    tcn = 4 * c + tloc
                    pv = ps_big.tile([128, 512], f32, tag="big")
                    for kc in range(KC):
                        nc.tensor.matmul(
                            pv[:, 0:192],
                            lhsT=xtb_c[:, kc, 128 * tloc : 128 * tloc + 128],
                            rhs=wv[:, kc, :],
                            start=(kc == 0),
                            stop=(kc == KC - 1),
                        )
                    nc.vector.tensor_copy(
                        out=v5[:, tcn, :, 0:64],
                        in_=pv[:, 0:192].rearrange("p (h e) -> p h e", e=64),
                    )

            def emit_scores_head(c, h, ets):
                i_base = 512 * c
                for p in range(2 * c + 2):
                    st = ps_st.tile([128, 1024], f32, tag="st")
                    # pairs 6/7 only exist for group 3 and are never live
                    # across two groups; single-buffer them to save SBUF
                    et = etp.tile(
                        [128, 1024],
                        bf16,
                        tag=f"et{h}_{p}",
                        name="et",
                        bufs=(2 if p < 6 else 1),
                    )
                    ets[(h, p)] = et
                    s0 = None
                    for half in range(2):
                        jc = 2 * p + half
                        i0 = 128 * (jc - 4 * c) if jc >= 4 * c else 0
                        if half == 0:
                            s0 = i0
                        lo = 512 * half
                        nc.tensor.matmul(
                            st[:, lo + i0 : lo + 512],
                            lhsT=kt[h][:, 128 * jc : 128 * jc + 128],
                            rhs=qk[h][:, i_base + i0 : i_base + 512],
                            start=True,
                            stop=True,
                        )
                    # one exp per pair tile; unwritten PSUM regions are
                    # only exp'd into et cols no y matmul ever reads
                    nc.scalar.activation(et[:, s0:1024], st[:, s0:1024], EXP)
                    for half in range(2):
                        jc = 2 * p + half
                        m = jc - 4 * c
                        if m >= 0:
                            # block-causal: queries of the lower half
                            # block can't see keys of the upper one
                            i0 = 512 * half + 128 * m
                            nc.vector.memset(et[64:128, i0 : i0 + 64], 0.0)

            def emit_y(c, ets):
                # y accumulation: r=0,1 in yA, r=2,3 in yB, head h at cols
                # 65h (+195 for odd r); col 65h+64 = softmax denominator.
                # Chains are kept atomic (consecutive matmuls) — interleaving
                # open PSUM accumulation chains with other matmuls breaks
                # the accumulation.
                yab = [
                    ps_yab.tile([128, 390], f32, tag="yA", name="yA"),
                    ps_yab.tile([128, 390], f32, tag="yB", name="yB"),
                ]
                # all chains first (PE never waits on the DVE normalize
                # round-trip), then the four norm+proj sequences pipeline
                for h in range(3):
                    for r in range(4):
                        cr = 4 * c + r
                        yt_ = yab[r // 2]
                        base = 195 * (r & 1)
                        for jc in range(cr + 1):
                            nc.tensor.matmul(
                                yt_[:, base + 65 * h : base + 65 * h + 65],
                                lhsT=ets[(h, jc // 2)][
                                    :,
                                    512 * (jc & 1) + 128 * r : 512 * (jc & 1)
                                    + 128 * r
                                    + 128,
                                ],
                                rhs=v_all[:, jc, 65 * h : 65 * h + 65],
                                start=(jc == 0),
                                stop=(jc == cr),
                            )
                for r in range(4):
                    emit_norm_proj(c, r, yab[r // 2])

            def emit_norm_proj(c, r, yt_):
                tcn = 4 * c + r
                tsl = slice(128 * tcn, 128 * tcn + 128)
                y3 = yt_.rearrange("p (g h e) -> p g h e", g=2, e=65)
                g = r & 1
                rec = small.tile([128, 3], f32, tag="rec")
                nc.vector.reciprocal(rec[:, :], y3[:, g, :, 64:65])
                for h in range(3):
                    nc.vector.tensor_scalar_mul(
                        y_all[:, tcn, 64 * h : 64 * h + 64],
                        y3[:, g, h, 0:64],
                        rec[:, h : h + 1],
                    )
                pt = ps_big.tile([128, 256], bf16, tag="big")
                nc.tensor.transpose(pt[:, 0:128], y_all[:, tcn, 0:128], id_bf)
                nc.tensor.transpose(pt[:, 128:256], y_all[:, tcn, 128:256], id_bf)
                nc.vector.tensor_copy(
                    out=yt_all[:, :, tsl],
                    in_=pt[:, :].rearrange("p (s t) -> p s t", s=2),
                )
                ot = outst.tile([128, C], bf16, tag="ot")
                for oc, ow in ((0, 512), (1, 256)):
                    pp = ps_big.tile([128, 512], f32, tag="big")
                    osl = slice(512 * oc, 512 * oc + ow)
                    nc.tensor.matmul(
                        pp[:, 0:ow],
                        lhsT=yt_all[:, 0, tsl],
                        rhs=wp[:, 0, osl],
                        start=True,
                        stop=False,
                    )
                    nc.tensor.matmul(
                        pp[:, 0:ow],
                        lhsT=yt_all[:, 1, tsl],
                        rhs=wp[:, 1, osl],
                        start=False,
                        stop=True,
                    )
                    # both copies on DVE: a waiting copy on the Scalar engine
                    # blocks ready EXPs behind it (engines are in-order)
                    nc.vector.tensor_copy(out=ot[:, osl], in_=pp[:, 0:ow])
                # out DMA on the gpsimd queue: its (late) data-ready wait must
                # not block the input loads streaming on the sync queue
                nc.gpsimd.dma_start(out=out_d[tsl, :], in_=ot[:, :])

            # software pipeline. Groups run in order 0,1,3,2 (the smaller
            # group-2 tail ends the kernel), qkv chunks are interleaved
            # between score-head phases, and each group's first score head is
            # prefetched before the previous group's y phase so the Scalar
            # engine never idles at group boundaries (et tiles double-buffer).
            ets_all = {c: {} for c in range(4)}
            # chunk-0 front: head h's scores need only qkv m-chunk h, so the
            # first exps fire right after m0 instead of after the full chunk
            emit_qkv_m(0, 0)
            emit_scores_head(0, 0, ets_all[0])
            emit_qkv_m(0, 1)
            emit_scores_head(0, 1, ets_all[0])
            emit_qkv_m(0, 2)
            emit_qkv_v(0)
            emit_scores_head(0, 2, ets_all[0])
            emit_qkv_qk(1)
            emit_qkv_v(1)
            order = [0, 1, 3, 2]
            for i, c in enumerate(order):
                nxt = order[i + 1] if i + 1 < 4 else None
                if nxt is not None:
                    emit_scores_head(nxt, 0, ets_all[nxt])
                emit_y(c, ets_all[c])
                if nxt is not None:
                    emit_scores_head(nxt, 1, ets_all[nxt])
                    if i == 0:
                        emit_qkv_qk(2)
                    if i == 1:
                        emit_qkv_v(2)
                    emit_scores_head(nxt, 2, ets_all[nxt])
                    if i == 0:
                        emit_qkv_qk(3)
                    if i == 1:
                        emit_qkv_v(3)

    nc.compile()
    return nc


def _get_nc():
    if "nc" not in _CACHE:
        _CACHE["nc"] = _build_bass()
    return _CACHE["nc"]


def make_in_maps(x, c_attn_w, c_proj_w, s):
    import ml_dtypes

    bf16 = ml_dtypes.bfloat16
    x = np.asarray(x, dtype=np.float32)
    c_attn_w = np.asarray(c_attn_w, dtype=np.float32)
    c_proj_w = np.asarray(c_proj_w, dtype=np.float32)
    s = np.asarray(s, dtype=np.float32)

    scale = np.float32(s[0] * np.log(T).astype(np.float32))
    f = np.float32(scale * np.float32(1.0 / np.sqrt(HD)))

    in_maps = []
    for b in range(2):
        xtb = np.ascontiguousarray(x[b].T).astype(bf16)  # [768, 2048]
        for g in range(4):
            hs = [3 * g, 3 * g + 1, 3 * g + 2]
            qrow = lambda h: c_attn_w[64 * h : 64 * h + 64] * f  # scaled q
            krow = lambda h: c_attn_w[C + 64 * h : C + 64 * h + 64]
            vrow = lambda h: c_attn_w[2 * C + 64 * h : 2 * C + 64 * h + 64]
            wqk = np.concatenate(
                [blk for h in hs for blk in (qrow(h), krow(h))], axis=0
            )  # [384, 768] rows = [q0,k0,q1,k1,q2,k2]
            wv = np.concatenate([vrow(h) for h in hs], axis=0)  # [192, 768]
            wp = np.zeros((256, C), np.float32)  # rows 192:256 stay zero
            wp[0:192] = c_proj_w[:, 192 * g : 192 * g + 192].T
            in_maps.append(
                {
                    "xtb": xtb,
                    "wqk": np.ascontiguousarray(wqk.T).astype(bf16),
                    "wv": np.ascontiguousarray(wv.T).astype(bf16),
                    "wp": wp.astype(bf16),
                }
            )
    return in_maps


def gather(results):
    out = np.empty((2, T, C), dtype=np.float32)
    for b in range(2):
        acc = results[4 * b]["out"].astype(np.float32)
        for g in range(1, 4):
            acc = acc + results[4 * b + g]["out"].astype(np.float32)
        out[b] = acc
    return out


def kernel(x, c_attn_w, c_proj_w, s):
    from concourse.bass_utils import run_bass_kernel_spmd

    nc = _get_nc()
    in_maps = make_in_maps(x, c_attn_w, c_proj_w, s)
    res = run_bass_kernel_spmd(nc, in_maps, list(range(N_CORES)))
    return gather(res.results)
